# revision 8
# baseline (speedup 1.0000x reference)
"""ALiBi multi-head attention with LoRA projections on 8 TRN2 NeuronCores.

Sharding: query-parallel. Core c handles batch b=c//4, query rows
[512*(c%4), 512*(c%4+1)) of that batch, all 16 heads.  K/V are computed
for the local 512 tokens and AllGathered (bf16) within each 4-core batch
group.  The non-causal ALiBi softmax factorizes as
    softmax(s_ij + slope*(j-i))_j = exp(s_ij) * E_j / sum_j exp(s_ij) * E_j,
      E_j = exp(slope*(j - (S-1)))
so no row-max / row-sum passes are needed: E is folded into V (and an
extra all-ones column of V yields the denominator as matmul output).
Heads with large slopes only attend to the last few key tiles
(contributions beyond that are < e^-20 relative) -> per-head key ranges.
"""

import os
import sys
import threading

import numpy as np
import ml_dtypes

sys.path.insert(0, "/opt/trn_rl_repo")

B, S, E, H, D, R = 2, 2048, 1024, 16, 64, 8
NC = 8
TQ = S // 4          # 512 tokens per core
KT = [1, 1, 1, 2, 4, 8] + [16] * 10   # key tiles (of 128) per head, ending at S
F32 = np.float32
BF16 = ml_dtypes.bfloat16

# per-kt V'' load groups: (kt0, nkt, hmin)
VG_GROUPS = [(0, 8, 6), (8, 4, 5), (12, 2, 4), (14, 1, 3), (15, 1, 0)]


def _kt_group(kt):
    for gi, (kt0, nkt, hmin) in enumerate(VG_GROUPS):
        if kt0 <= kt < kt0 + nkt:
            return gi, kt - kt0, hmin
    raise AssertionError(kt)


_BUILT = None
_LOCK = threading.Lock()


def _build():
    import concourse.bass as bass
    import concourse.tile as tile
    from concourse import bacc, mybir

    f32 = mybir.dt.float32
    bf16 = mybir.dt.bfloat16
    AF = mybir.ActivationFunctionType
    ALU = mybir.AluOpType

    nc = bacc.Bacc(
        "TRN2", target_bir_lowering=False, debug=False,
        enable_asserts=False, num_devices=NC,
    )

    def din(name, shape, dt):
        return nc.dram_tensor(name, shape, dt, kind="ExternalInput").ap()

    xT = din("xT", [E, TQ], f32)
    WT = {n: din(f"W{n}T", [E, E], bf16) for n in "qkvo"}
    AT = din("AT", [E, 27], bf16)
    AoT = din("AoT", [E, R], bf16)
    BALL = din("BALL", [128, E], bf16)
    BoT = din("BoT", [9, E], bf16)
    ETd = din("ET", [TQ, H], f32)
    rzd = din("rz", [128, 1], f32)
    out_d = nc.dram_tensor("out", [E, TQ], f32, kind="ExternalOutput").ap()

    with tile.TileContext(nc) as tc:
        import contextlib
        ctx = contextlib.ExitStack()
        dram = ctx.enter_context(tc.tile_pool(name="dram", bufs=1, space="DRAM"))
        kin = dram.tile([E, TQ], bf16)
        kg = dram.tile([4 * E, TQ], bf16)
        vin = dram.tile([TQ, H * 65], bf16)
        vg = dram.tile([S, H * 65], bf16)

        cpool = ctx.enter_context(tc.tile_pool(name="consts", bufs=1))
        wpool = ctx.enter_context(tc.tile_pool(name="work", bufs=1))
        ppool = ctx.enter_context(tc.tile_pool(name="ptiles", bufs=3))
        spool = ctx.enter_context(tc.tile_pool(name="small", bufs=2))
        # one PSUM pool, 8 banks total: tag "big" ([128,1024] f32 = 2 banks,
        # 3 bufs = 6 banks) shared by proj/t1/scores/bcast; tag "ot"
        # ([65,512] = 1 bank, 2 bufs) for the per-head PV accumulators.
        psum = ctx.enter_context(tc.tile_pool(name="psum", bufs=3, space="PSUM"))

        # ---- constant loads ----
        x_f = cpool.tile([128, 8, TQ], f32, name="x_f")
        nc.sync.dma_start(x_f[:], xT.rearrange("(k p) t -> p k t", p=128))
        W_sb = {}
        for n in "qkvo":
            W_sb[n] = cpool.tile([128, 8, E], bf16, name=f"W{n}_sb")
            nc.sync.dma_start(W_sb[n][:], WT[n].rearrange("(k p) m -> p k m", p=128))
        AT_sb = cpool.tile([128, 8, 27], bf16, name="AT_sb")
        nc.sync.dma_start(AT_sb[:], AT.rearrange("(k p) m -> p k m", p=128))
        AoT_sb = cpool.tile([128, 8, R], bf16, name="AoT_sb")
        nc.sync.dma_start(AoT_sb[:], AoT.rearrange("(k p) m -> p k m", p=128))
        Ball_sb = cpool.tile([128, E], bf16, name="Ball_sb")
        nc.sync.dma_start(Ball_sb[:], BALL[:, :])
        Bo_sb = cpool.tile([9, E], bf16, name="Bo_sb")
        nc.sync.dma_start(Bo_sb[:], BoT[:, :])
        ET_sb = cpool.tile([128, 4, H], f32, name="ET_sb")
        nc.sync.dma_start(ET_sb[:], ETd.rearrange("(tt p) h -> p tt h", p=128))
        rz_sb = cpool.tile([128, 1], f32, name="rz_sb")
        nc.sync.dma_start(rz_sb[:], rzd[:, :])
        ones1 = cpool.tile([1, 64], f32, name="ones1")
        nc.vector.memset(ones1[:], 1.0)
        ones512 = cpool.tile([1, TQ], bf16, name="ones512")
        nc.vector.memset(ones512[:], 1.0)
        e8 = cpool.tile([1, 9], bf16, name="e8")
        nc.vector.memset(e8[:], 0.0)
        nc.vector.memset(e8[:, 8:9], 1.0)

        # warm the ACT exp table early (table load ~2.7us overlaps with DMAs)
        warm = cpool.tile([1, 16], f32, name="warm")
        nc.vector.memset(warm[:], 0.0)
        nc.scalar.activation(warm[:], warm[:], AF.Exp)

        # ---- cast x to bf16 ----
        x_b = wpool.tile([128, 8, TQ], bf16, name="x_b", tag="xb_ot")
        nc.vector.tensor_copy(x_b[:, 0:4, :], x_f[:, 0:4, :])
        nc.vector.tensor_copy(x_b[:, 4:8, :], x_f[:, 4:8, :])

        # ---- t1 = A @ x.T (lora down-proj for q,k,v) + ones rows ----
        # row groups at partition bases 0 (q), 32 (k), 64 (v) so the rank-9
        # B-matmuls satisfy the base-partition-match constraint.
        ps_t1 = psum.tile([73, TQ], f32, tag="big", name="ps_t1")
        for gi, c0 in ((0, 0), (1, 9), (2, 18)):
            # ones row (partition 32*gi+8) via e8 x ones512, then lora-A accum
            nc.tensor.matmul(ps_t1[32 * gi:32 * gi + 9, :], e8[:],
                             ones512[:], start=True, stop=False)
            for k in range(8):
                nc.tensor.matmul(ps_t1[32 * gi:32 * gi + 8, :],
                                 AT_sb[:, k, c0:c0 + 8], x_b[:, k, :],
                                 start=False, stop=(k == 7))
        t1 = wpool.tile([128, TQ], bf16, name="t1")
        for gi in range(3):
            nc.vector.tensor_copy(t1[32 * gi:32 * gi + 9, :],
                                  ps_t1[32 * gi:32 * gi + 9, :])

        # ---- K projection (transposed layout [d, tok]) ----
        Kloc = wpool.tile([128, 8, TQ], bf16, name="Kloc", tag="kq")
        for m in range(8):
            ps = psum.tile([128, TQ], f32, tag="big", name="ps_proj")
            for k in range(8):
                nc.tensor.matmul(ps[:], W_sb["k"][:, k, m * 128:(m + 1) * 128],
                                 x_b[:, k, :], start=(k == 0), stop=False)
            nc.tensor.matmul(ps[:], Ball_sb[32:41, m * 128:(m + 1) * 128],
                             t1[32:41, :], start=False, stop=True)
            nc.vector.tensor_copy(Kloc[:, m, :], ps[:])
        nc.sync.dma_start(kin.rearrange("(m p) t -> p m t", p=128), Kloc[:])
        nc.gpsimd.collective_compute(
            "AllGather", mybir.AluOpType.bypass,
            replica_groups=[[0, 1, 2, 3], [4, 5, 6, 7]],
            ins=[kin.opt()], outs=[kg.opt()],
        )

        # ---- V projection (natural layout [tok, d]), E-scaled, + E columns ----
        V2 = wpool.tile([128, 4, H * 65], bf16, name="V2", tag="v2")
        for tt in range(4):
            for nh in range(2):
                ps = psum.tile([128, 512], f32, tag="big", name="ps_projv")
                for k in range(8):
                    nc.tensor.matmul(ps[:], x_b[:, k, tt * 128:(tt + 1) * 128],
                                     W_sb["v"][:, k, nh * 512:(nh + 1) * 512],
                                     start=(k == 0), stop=False)
                nc.tensor.matmul(ps[:], t1[64:73, tt * 128:(tt + 1) * 128],
                                 Ball_sb[64:73, nh * 512:(nh + 1) * 512],
                                 start=False, stop=True)
                for hh in range(8):
                    h = nh * 8 + hh
                    nc.vector.tensor_scalar_mul(
                        V2[:, tt, h * 65:h * 65 + 64],
                        ps[:, hh * 64:(hh + 1) * 64],
                        ET_sb[:, tt, h:h + 1])
            # E columns (denominator): V2[:, tt, h*65+64] = E
            nc.vector.tensor_copy(V2[:, tt, 64:H * 65:65], ET_sb[:, tt, :])
        nc.sync.dma_start(vin.rearrange("(tt p) c -> p tt c", p=128), V2[:])
        nc.gpsimd.collective_compute(
            "AllGather", mybir.AluOpType.bypass,
            replica_groups=[[0, 1, 2, 3], [4, 5, 6, 7]],
            ins=[vin.opt()], outs=[vg.opt()],
        )

        # ---- Q projection (transposed layout [d, q]) ----
        Q_sb = wpool.tile([128, 8, TQ], bf16, name="Q_sb", tag="q")
        for m in range(8):
            ps = psum.tile([128, TQ], f32, tag="big", name="ps_proj")
            for k in range(8):
                nc.tensor.matmul(ps[:], W_sb["q"][:, k, m * 128:(m + 1) * 128],
                                 x_b[:, k, :], start=(k == 0), stop=False)
            nc.tensor.matmul(ps[:], Ball_sb[0:9, m * 128:(m + 1) * 128],
                             t1[0:9, :], start=False, stop=True)
            nc.vector.tensor_copy(Q_sb[:, m, :], ps[:])

        # ---- load gathered K (per d-pair, only needed key range) ----
        kgv = kg.rearrange("(r d p) t -> p r d t", d=8, p=128)
        Ksb = []
        for dp in range(8):
            T = KT[2 * dp + 1]
            t = cpool.tile([128, T * 128], bf16, name=f"Ksb{dp}")
            if T >= 4:
                nr = T // 4
                src = kgv[:, 4 - nr:4, dp, :]
                dst = t.rearrange("p (r t) -> p r t", t=512)
            else:
                tw = T * 128
                src = kgv[:, 3, dp, 512 - tw:512]
                dst = t[:]
            nc.sync.dma_start(dst, src)
            Ksb.append(t)

        # ---- load gathered V'' (per kt group, only needed head tail) ----
        vgv = vg.rearrange("(kt p) c -> p kt c", p=128)
        Vg = []
        for (kt0, nkt, hmin) in VG_GROUPS:
            c0 = 65 * hmin
            t = cpool.tile([128, nkt, H * 65 - c0], bf16, name=f"Vg{kt0}")
            nc.sync.dma_start(t[:], vgv[:, kt0:kt0 + nkt, c0:])
            Vg.append(t)

        def v2slice(kt, h):
            gi, ki, hmin = _kt_group(kt)
            c = (h - hmin) * 65
            return Vg[gi][:, ki, c:c + 65]

        # ---- attention ----
        OT = wpool.tile([128, 8, TQ], bf16, name="OT", tag="xb_ot")
        for hp in range(8):
            h0, h1 = 2 * hp, 2 * hp + 1
            T0, T1 = KT[h0], KT[h1]
            psO = [psum.tile([65, TQ], f32, tag="ot", bufs=2, name=f"psO{hp}_{i}")
                   for i in range(2)]
            first = [True, True]
            for kt in range(16 - T1, 16):
                paired = kt >= 16 - T0
                koff = kt - (16 - T1)
                ps = psum.tile([128, 1024], f32, tag="big", name=f"psS{hp}_{kt}")
                # h1 scores -> [:, 0:512] (contraction partitions 64..127)
                nc.tensor.matmul(ps[:, 0:512],
                                 Ksb[hp][64:128, koff * 128:(koff + 1) * 128],
                                 Q_sb[64:128, hp, :], start=True, stop=True)
                if paired:
                    nc.tensor.matmul(ps[:, 512:1024],
                                     Ksb[hp][0:64, koff * 128:(koff + 1) * 128],
                                     Q_sb[0:64, hp, :], start=True, stop=True)
                P = ppool.tile([128, 1024], bf16, tag="p", name=f"P{hp}_{kt}")
                if paired:
                    nc.scalar.activation(P[:], ps[:], AF.Exp)
                else:
                    nc.scalar.activation(P[:, 0:512], ps[:, 0:512], AF.Exp)
                nc.tensor.matmul(psO[1][:], v2slice(kt, h1), P[:, 0:512],
                                 start=first[1], stop=(kt == 15))
                first[1] = False
                if paired:
                    nc.tensor.matmul(psO[0][:], v2slice(kt, h0), P[:, 512:1024],
                                     start=first[0], stop=(kt == 15))
                    first[0] = False
            for i, h in ((0, h0), (1, h1)):
                rec = spool.tile([1, TQ], f32, tag="rec", name=f"rec{hp}_{i}")
                nc.vector.reciprocal(rec[:], psO[i][64:65, :])
                onum = spool.tile([64, TQ], bf16, tag="onum", name=f"on{hp}_{i}")
                nc.vector.tensor_copy(onum[:], psO[i][0:64, :])
                bc = psum.tile([64, TQ], f32, tag="big", name=f"bc{hp}_{i}")
                nc.tensor.matmul(bc[:], ones1[:], rec[:], start=True, stop=True)
                nc.vector.tensor_mul(OT[64 * i:64 * i + 64, hp, :], onum[:], bc[:])

        # ---- lora-o down-proj ----
        ps_t2 = psum.tile([9, TQ], f32, tag="big", name="ps_t2")
        nc.tensor.matmul(ps_t2[:], e8[:], ones512[:], start=True, stop=False)
        for k in range(8):
            nc.tensor.matmul(ps_t2[0:8, :], AoT_sb[:, k, :], OT[:, k, :],
                             start=False, stop=(k == 7))
        t2 = wpool.tile([9, TQ], bf16, name="t2")
        nc.vector.tensor_copy(t2[:], ps_t2[:])

        # ---- O projection + rezero residual ----
        out_sb = wpool.tile([128, 8, TQ], f32, name="out_sb", tag="osb")
        for m in range(8):
            ps = psum.tile([128, TQ], f32, tag="big", name="ps_proj")
            for k in range(8):
                nc.tensor.matmul(ps[:], W_sb["o"][:, k, m * 128:(m + 1) * 128],
                                 OT[:, k, :], start=(k == 0), stop=False)
            nc.tensor.matmul(ps[:], Bo_sb[:, m * 128:(m + 1) * 128],
                             t2[:], start=False, stop=True)
            nc.vector.scalar_tensor_tensor(
                out_sb[:, m, :], ps[:], rz_sb[:, 0:1], x_f[:, m, :],
                op0=ALU.mult, op1=ALU.add)
        nc.sync.dma_start(out_d.rearrange("(m p) t -> p m t", p=128), out_sb[:])
        ctx.close()

    if not os.environ.get("BASS_SKIP_COMPILE"):
        nc.compile()
    return nc


def _get_built():
    global _BUILT
    with _LOCK:
        if _BUILT is None:
            _BUILT = _build()
    return _BUILT


def _prep_inputs(inputs):
    """Host-side sharding + weight relayout. Returns in_maps for 8 cores."""
    x = np.asarray(inputs["x"], F32)
    rez = float(np.asarray(inputs["rezero"]).reshape(-1)[0])

    def bf(a):
        return np.ascontiguousarray(a.astype(BF16))

    WqT = bf(np.asarray(inputs["Wq"], F32).T / 8.0)
    WkT = bf(np.asarray(inputs["Wk"], F32).T)
    WvT = bf(np.asarray(inputs["Wv"], F32).T)
    WoT = bf(np.asarray(inputs["Wo"], F32).T)
    AT = np.zeros((E, 27), F32)
    AT[:, 0:8] = np.asarray(inputs["Aq"], F32).T
    AT[:, 9:17] = np.asarray(inputs["Ak"], F32).T
    AT[:, 18:26] = np.asarray(inputs["Av"], F32).T
    AT = bf(AT)
    AoT = bf(np.asarray(inputs["Ao"], F32).T)

    def baug(Bm, bias, scale):
        out = np.zeros((9, E), F32)
        out[0:8] = np.asarray(Bm, F32).T * scale
        out[8] = np.asarray(bias, F32) * (scale * 8.0)
        return out

    BALLf = np.zeros((128, E), F32)
    BALLf[0:9] = baug(inputs["Bq"], inputs["bq"], 1.0 / 64.0)
    BALLf[32:41] = baug(inputs["Bk"], inputs["bk"], 1.0 / 8.0)
    BALLf[64:73] = baug(inputs["Bv"], inputs["bv"], 1.0 / 8.0)
    BALL = bf(BALLf)
    BoT = bf(baug(inputs["Bo"], inputs["bo"], 1.0 / 8.0))

    slopes = 0.5 ** np.arange(H, dtype=F32)
    jpos = np.arange(S, dtype=F32)
    Efull = np.exp(slopes[:, None] * (jpos[None, :] - (S - 1))).astype(F32)  # [H,S]
    rz_vec = np.full((128, 1), rez, F32)

    in_maps = []
    for c in range(NC):
        b, r = c // 4, c % 4
        sl = slice(TQ * r, TQ * (r + 1))
        m = {
            "xT": np.ascontiguousarray(x[b, sl, :].T),
            "WqT": WqT, "WkT": WkT, "WvT": WvT, "WoT": WoT,
            "AT": AT, "AoT": AoT,
            "BALL": BALL, "BoT": BoT,
            "ET": np.ascontiguousarray(Efull[:, sl].T),
            "rz": rz_vec,
        }
        in_maps.append(m)
    return in_maps


def kernel(**inputs) -> np.ndarray:
    from concourse import bass_utils

    nc = _get_built()
    in_maps = _prep_inputs(inputs)
    res = bass_utils.run_bass_kernel_spmd(nc, in_maps, core_ids=list(range(NC)))
    out = np.zeros((B, S, E), F32)
    for c in range(NC):
        b, r = c // 4, c % 4
        out[b, TQ * r:TQ * (r + 1), :] = res.results[c]["out"].T
    return out


if __name__ == "__main__":
    _get_built()
    print("build+compile OK")


# revision 15
# speedup vs baseline: 1.3531x; 1.3531x over previous
"""ALiBi multi-head attention with LoRA projections on 8 TRN2 NeuronCores.

Sharding: query-parallel. Core c handles batch b=c//4, query rows
[512*(c%4), 512*(c%4+1)) of that batch, all 16 heads.  K/V are computed
for the local 512 tokens and AllGathered (bf16) within each 4-core batch
group.  The non-causal ALiBi softmax factorizes as
    softmax(s_ij + slope*(j-i))_j = exp(s_ij) * E_j / sum_j exp(s_ij) * E_j,
      E_j = exp(slope*(j - (S-1)))
so no row-max / row-sum passes are needed: E is folded into V (and an
extra all-ones column of V yields the denominator as matmul output).
Heads with large slopes only attend to the last few key tiles
(contributions beyond that are < ~e^-19 relative) -> per-head key ranges.

The attention loop is software-pipelined: scores+exp run LOOKAHEAD units
ahead of the PV matmuls, so ScalarE exp work fills the AllGather-V window.
"""

import os
import sys
import threading

import numpy as np
import ml_dtypes

sys.path.insert(0, "/opt/trn_rl_repo")

B, S, E, H, D, R = 2, 2048, 1024, 16, 64, 8
NC = 8
TQ = S // 4          # 512 tokens per core
# key tiles (of 128) per head, ranges end at S; ~22*2^h keys needed
KT = [1, 1, 1, 2, 3, 6, 12] + [16] * 9
# per-dp K-load tile count (rank-aligned roundup of KT[2dp+1])
TL = [1, 2, 8, 16, 16, 16, 16, 16]
F32 = np.float32
BF16 = ml_dtypes.bfloat16
LOOKAHEAD = 12

# per-kt V'' load groups: (kt0, nkt, hmin)
VG_GROUPS = [(0, 4, 7), (4, 6, 6), (10, 3, 5), (13, 1, 4), (14, 1, 3), (15, 1, 0)]


def _kt_group(kt):
    for gi, (kt0, nkt, hmin) in enumerate(VG_GROUPS):
        if kt0 <= kt < kt0 + nkt:
            return gi, kt - kt0, hmin
    raise AssertionError(kt)


_BUILT = None
_LOCK = threading.Lock()


def _build():
    import concourse.bass as bass
    import concourse.tile as tile
    from concourse import bacc, mybir

    f32 = mybir.dt.float32
    bf16 = mybir.dt.bfloat16
    AF = mybir.ActivationFunctionType
    ALU = mybir.AluOpType

    nc = bacc.Bacc(
        "TRN2", target_bir_lowering=False, debug=False,
        enable_asserts=False, num_devices=NC,
    )

    def din(name, shape, dt):
        return nc.dram_tensor(name, shape, dt, kind="ExternalInput").ap()

    xT = din("xT", [E, TQ], f32)
    WT = {n: din(f"W{n}T", [E, E], bf16) for n in "qkvo"}
    AT = din("AT", [E, 27], bf16)
    AoT = din("AoT", [E, R], bf16)
    BALL = din("BALL", [128, E], bf16)
    BoT = din("BoT", [9, E], bf16)
    ETd = din("ET", [TQ, H], f32)
    rzd = din("rz", [128, 1], f32)
    out_d = nc.dram_tensor("out", [E, TQ], f32, kind="ExternalOutput").ap()

    with tile.TileContext(nc) as tc:
        import contextlib
        ctx = contextlib.ExitStack()
        dram = ctx.enter_context(tc.tile_pool(name="dram", bufs=1, space="DRAM"))
        kin = dram.tile([E, TQ], bf16)
        kg = dram.tile([4 * E, TQ], bf16)
        vin = dram.tile([TQ, H * 65], bf16)
        vg = dram.tile([S, H * 65], bf16)

        cpool = ctx.enter_context(tc.tile_pool(name="consts", bufs=1))
        wpool = ctx.enter_context(tc.tile_pool(name="work", bufs=1))
        ppool = ctx.enter_context(tc.tile_pool(name="ptiles", bufs=LOOKAHEAD + 2))
        spool = ctx.enter_context(tc.tile_pool(name="small", bufs=4))
        # one PSUM pool, 8 banks: tag "big" ([128,1024] f32 = 2 banks, 3 bufs)
        # shared by proj/t1/scores/bcast; tag "ot" ([65,512] = 1 bank, 2 bufs).
        psum = ctx.enter_context(tc.tile_pool(name="psum", bufs=3, space="PSUM"))

        # ---- phase 0: loads needed for K path first ----
        x_f = cpool.tile([128, 8, TQ], f32, name="x_f")
        nc.sync.dma_start(x_f[:], xT.rearrange("(k p) t -> p k t", p=128))
        AT_sb = cpool.tile([128, 8, 27], bf16, name="AT_sb")
        nc.sync.dma_start(AT_sb[:], AT.rearrange("(k p) m -> p k m", p=128))
        Ball_sb = cpool.tile([128, E], bf16, name="Ball_sb")
        nc.sync.dma_start(Ball_sb[:], BALL[:, :])
        W_sb = {}
        for n in "kvq":   # K first: its AllGather is on the critical path
            W_sb[n] = wpool.tile([128, 8, E], bf16, name=f"W{n}_sb",
                                 tag=("wk_wo" if n == "k" else f"w{n}"))
            nc.sync.dma_start(W_sb[n][:], WT[n].rearrange("(k p) m -> p k m", p=128))
        ET_sb = cpool.tile([128, 4, H], f32, name="ET_sb")
        nc.sync.dma_start(ET_sb[:], ETd.rearrange("(tt p) h -> p tt h", p=128))

        ones1 = cpool.tile([1, 64], bf16, name="ones1")
        nc.vector.memset(ones1[:], 1.0)
        ones512 = cpool.tile([1, TQ], bf16, name="ones512")
        nc.vector.memset(ones512[:], 1.0)
        e8 = cpool.tile([1, 9], bf16, name="e8")
        nc.vector.memset(e8[:], 0.0)
        nc.vector.memset(e8[:, 8:9], 1.0)

        # warm the ACT exp table early (table load ~2.7us overlaps with DMAs)
        warm = cpool.tile([1, 16], f32, name="warm")
        nc.vector.memset(warm[:], 0.0)
        nc.scalar.activation(warm[:], warm[:], AF.Exp)

        # ---- cast x to bf16 ----
        x_b = wpool.tile([128, 8, TQ], bf16, name="x_b", tag="xb_ot")
        nc.vector.tensor_copy(x_b[:, 0:4, :], x_f[:, 0:4, :])
        nc.vector.tensor_copy(x_b[:, 4:8, :], x_f[:, 4:8, :])

        # ---- t1 = lora-A down-proj for q,k,v; row groups at bases 0/32/64
        # with a trailing all-ones row each (via e8 x ones matmul) ----
        ps_t1 = psum.tile([73, TQ], f32, tag="big", name="ps_t1")
        for gi, c0 in ((1, 9), (0, 0), (2, 18)):   # k group first
            nc.tensor.matmul(ps_t1[32 * gi:32 * gi + 9, :], e8[:],
                             ones512[:], start=True, stop=False)
            for k in range(8):
                nc.tensor.matmul(ps_t1[32 * gi:32 * gi + 8, :],
                                 AT_sb[:, k, c0:c0 + 8], x_b[:, k, :],
                                 start=False, stop=(k == 7))
        t1 = wpool.tile([128, TQ], bf16, name="t1")
        for gi in range(3):
            nc.vector.tensor_copy(t1[32 * gi:32 * gi + 9, :],
                                  ps_t1[32 * gi:32 * gi + 9, :])

        # ---- K projection (transposed layout [d, tok]) + AllGather ----
        Kloc = wpool.tile([128, 8, TQ], bf16, name="Kloc", tag="kq")
        for m in range(8):
            ps = psum.tile([128, TQ], f32, tag="big", name="ps_proj")
            for k in range(8):
                nc.tensor.matmul(ps[:], W_sb["k"][:, k, m * 128:(m + 1) * 128],
                                 x_b[:, k, :], start=(k == 0), stop=False)
            nc.tensor.matmul(ps[:], Ball_sb[32:41, m * 128:(m + 1) * 128],
                             t1[32:41, :], start=False, stop=True)
            nc.vector.tensor_copy(Kloc[:, m, :], ps[:])
        nc.sync.dma_start(kin.rearrange("(m p) t -> p m t", p=128), Kloc[:])
        nc.gpsimd.collective_compute(
            "AllGather", mybir.AluOpType.bypass,
            replica_groups=[[0, 1, 2, 3], [4, 5, 6, 7]],
            ins=[kin.opt()], outs=[kg.opt()],
        )

        # ---- V projection (natural layout [tok, d]), E-scaled, + E columns ----
        V2 = wpool.tile([128, 4, H * 65], bf16, name="V2", tag="v2")
        for tt in range(4):
            for nh in range(2):
                ps = psum.tile([128, 512], f32, tag="big", name="ps_projv")
                for k in range(8):
                    nc.tensor.matmul(ps[:], x_b[:, k, tt * 128:(tt + 1) * 128],
                                     W_sb["v"][:, k, nh * 512:(nh + 1) * 512],
                                     start=(k == 0), stop=False)
                nc.tensor.matmul(ps[:], t1[64:73, tt * 128:(tt + 1) * 128],
                                 Ball_sb[64:73, nh * 512:(nh + 1) * 512],
                                 start=False, stop=True)
                for hh in range(8):
                    h = nh * 8 + hh
                    nc.vector.tensor_scalar_mul(
                        V2[:, tt, h * 65:h * 65 + 64],
                        ps[:, hh * 64:(hh + 1) * 64],
                        ET_sb[:, tt, h:h + 1])
            nc.vector.tensor_copy(V2[:, tt, 64:H * 65:65], ET_sb[:, tt, :])
        nc.sync.dma_start(vin.rearrange("(tt p) c -> p tt c", p=128), V2[:])
        nc.gpsimd.collective_compute(
            "AllGather", mybir.AluOpType.bypass,
            replica_groups=[[0, 1, 2, 3], [4, 5, 6, 7]],
            ins=[vin.opt()], outs=[vg.opt()],
        )

        # ---- Q projection (transposed layout [d, q]) ----
        Q_sb = wpool.tile([128, 8, TQ], bf16, name="Q_sb", tag="kq")
        for m in range(8):
            ps = psum.tile([128, TQ], f32, tag="big", name="ps_proj")
            for k in range(8):
                nc.tensor.matmul(ps[:], W_sb["q"][:, k, m * 128:(m + 1) * 128],
                                 x_b[:, k, :], start=(k == 0), stop=False)
            nc.tensor.matmul(ps[:], Ball_sb[0:9, m * 128:(m + 1) * 128],
                             t1[0:9, :], start=False, stop=True)
            nc.vector.tensor_copy(Q_sb[:, m, :], ps[:])

        # ---- load gathered K (per d-pair, rank-aligned key ranges) ----
        kgv = kg.rearrange("(r d p) t -> p r d t", d=8, p=128)
        Ksb = []
        for dp in range(8):
            T = TL[dp]
            t = cpool.tile([128, T * 128], bf16, name=f"Ksb{dp}")
            if T >= 4:
                nr = T // 4
                src = kgv[:, 4 - nr:4, dp, :]
                dst = t.rearrange("p (r t) -> p r t", t=512)
            else:
                tw = T * 128
                src = kgv[:, 3, dp, 512 - tw:512]
                dst = t[:]
            nc.sync.dma_start(dst, src)
            Ksb.append(t)

        # ---- load gathered V'' (per kt group, only needed head tail) ----
        vgv = vg.rearrange("(kt p) c -> p kt c", p=128)
        Vg = []
        for (kt0, nkt, hmin) in VG_GROUPS:
            c0 = 65 * hmin
            t = cpool.tile([128, nkt, H * 65 - c0], bf16, name=f"Vg{kt0}")
            nc.sync.dma_start(t[:], vgv[:, kt0:kt0 + nkt, c0:])
            Vg.append(t)

        def v2slice(kt, h):
            gi, ki, hmin = _kt_group(kt)
            c = (h - hmin) * 65
            return Vg[gi][:, ki, c:c + 65]

        # ---- attention, software-pipelined: scores+exp LOOKAHEAD units
        # ahead of PV so exp fills the AllGather-V window ----
        OT = wpool.tile([128, 8, TQ], bf16, name="OT", tag="xb_ot")
        units = []
        for hp in range(8):
            T1 = KT[2 * hp + 1]
            for kt in range(16 - T1, 16):
                units.append((hp, kt))
        nU = len(units)
        Pt = {}
        psO = {}
        first = {}
        for step in range(nU + LOOKAHEAD):
            if step < nU:
                hp, kt = units[step]
                T0 = KT[2 * hp]
                paired = kt >= 16 - T0
                koff = kt - (16 - TL[hp])
                ps = psum.tile([128, 1024], f32, tag="big", name=f"psS{step}")
                nc.tensor.matmul(ps[:, 0:512],
                                 Ksb[hp][64:128, koff * 128:(koff + 1) * 128],
                                 Q_sb[64:128, hp, :], start=True, stop=True)
                if paired:
                    nc.tensor.matmul(ps[:, 512:1024],
                                     Ksb[hp][0:64, koff * 128:(koff + 1) * 128],
                                     Q_sb[0:64, hp, :], start=True, stop=True)
                P = ppool.tile([128, 1024], bf16, tag="p", name=f"P{step}")
                if paired:
                    nc.scalar.activation(P[:], ps[:], AF.Exp)
                else:
                    nc.scalar.activation(P[:, 0:512], ps[:, 0:512], AF.Exp)
                Pt[step] = P
            j = step - LOOKAHEAD
            if j < 0:
                continue
            hp, kt = units[j]
            T0 = KT[2 * hp]
            paired = kt >= 16 - T0
            if hp not in psO:
                psO[hp] = [psum.tile([65, TQ], f32, tag="ot", bufs=2,
                                     name=f"psO{hp}_{i}") for i in range(2)]
                first[hp] = [True, True]
            P = Pt.pop(j)
            nc.tensor.matmul(psO[hp][1][:], v2slice(kt, 2 * hp + 1), P[:, 0:512],
                             start=first[hp][1], stop=(kt == 15))
            first[hp][1] = False
            if paired:
                nc.tensor.matmul(psO[hp][0][:], v2slice(kt, 2 * hp), P[:, 512:1024],
                                 start=first[hp][0], stop=(kt == 15))
                first[hp][0] = False
            if kt == 15:
                for i in range(2):
                    lsb = spool.tile([1, TQ], f32, tag="lsb", bufs=2, name=f"l{hp}_{i}")
                    nc.vector.tensor_copy(lsb[:], psO[hp][i][64:65, :])
                    recf = spool.tile([1, TQ], f32, tag="recf", bufs=2, name=f"rf{hp}_{i}")
                    nc.vector.reciprocal_approx_fast(recf[:], lsb[:])
                    rec = spool.tile([1, TQ], bf16, tag="rec", bufs=2, name=f"rec{hp}_{i}")
                    nc.vector.tensor_copy(rec[:], recf[:])
                    onum = spool.tile([64, TQ], bf16, tag="onum", bufs=2, name=f"on{hp}_{i}")
                    nc.vector.tensor_copy(onum[:], psO[hp][i][0:64, :])
                    bc = psum.tile([64, TQ], f32, tag="big", name=f"bc{hp}_{i}")
                    nc.tensor.matmul(bc[:], ones1[:], rec[:], start=True, stop=True)
                    nc.vector.tensor_mul(OT[64 * i:64 * i + 64, hp, :], onum[:], bc[:])
                del psO[hp]

        # ---- late consts for the O path (Wo reuses Wk's SBUF slot) ----
        W_sb["o"] = wpool.tile([128, 8, E], bf16, name="Wo_sb", tag="wk_wo")
        nc.sync.dma_start(W_sb["o"][:], WT["o"].rearrange("(k p) m -> p k m", p=128))
        AoT_sb = cpool.tile([128, 8, R], bf16, name="AoT_sb")
        nc.sync.dma_start(AoT_sb[:], AoT.rearrange("(k p) m -> p k m", p=128))
        Bo_sb = cpool.tile([9, E], bf16, name="Bo_sb")
        nc.sync.dma_start(Bo_sb[:], BoT[:, :])
        rz_sb = cpool.tile([128, 1], f32, name="rz_sb")
        nc.sync.dma_start(rz_sb[:], rzd[:, :])

        # ---- lora-o down-proj ----
        ps_t2 = psum.tile([9, TQ], f32, tag="big", name="ps_t2")
        nc.tensor.matmul(ps_t2[:], e8[:], ones512[:], start=True, stop=False)
        for k in range(8):
            nc.tensor.matmul(ps_t2[0:8, :], AoT_sb[:, k, :], OT[:, k, :],
                             start=False, stop=(k == 7))
        t2 = wpool.tile([9, TQ], bf16, name="t2")
        nc.vector.tensor_copy(t2[:], ps_t2[:])

        # ---- O projection + rezero residual ----
        out_sb = wpool.tile([128, 8, TQ], f32, name="out_sb", tag="v2")
        for m in range(8):
            ps = psum.tile([128, TQ], f32, tag="big", name="ps_proj")
            for k in range(8):
                nc.tensor.matmul(ps[:], W_sb["o"][:, k, m * 128:(m + 1) * 128],
                                 OT[:, k, :], start=(k == 0), stop=False)
            nc.tensor.matmul(ps[:], Bo_sb[:, m * 128:(m + 1) * 128],
                             t2[:], start=False, stop=True)
            nc.vector.scalar_tensor_tensor(
                out_sb[:, m, :], ps[:], rz_sb[:, 0:1], x_f[:, m, :],
                op0=ALU.mult, op1=ALU.add)
        nc.sync.dma_start(out_d.rearrange("(m p) t -> p m t", p=128), out_sb[:])
        ctx.close()

    if not os.environ.get("BASS_SKIP_COMPILE"):
        nc.compile()
    return nc


def _get_built():
    global _BUILT
    with _LOCK:
        if _BUILT is None:
            _BUILT = _build()
    return _BUILT


def _prep_inputs(inputs):
    """Host-side sharding + weight relayout. Returns in_maps for 8 cores."""
    x = np.asarray(inputs["x"], F32)
    rez = float(np.asarray(inputs["rezero"]).reshape(-1)[0])

    def bf(a):
        return np.ascontiguousarray(a.astype(BF16))

    WqT = bf(np.asarray(inputs["Wq"], F32).T / 8.0)
    WkT = bf(np.asarray(inputs["Wk"], F32).T)
    WvT = bf(np.asarray(inputs["Wv"], F32).T)
    WoT = bf(np.asarray(inputs["Wo"], F32).T)
    AT = np.zeros((E, 27), F32)
    AT[:, 0:8] = np.asarray(inputs["Aq"], F32).T
    AT[:, 9:17] = np.asarray(inputs["Ak"], F32).T
    AT[:, 18:26] = np.asarray(inputs["Av"], F32).T
    AT = bf(AT)
    AoT = bf(np.asarray(inputs["Ao"], F32).T)

    def baug(Bm, bias, scale):
        out = np.zeros((9, E), F32)
        out[0:8] = np.asarray(Bm, F32).T * scale
        out[8] = np.asarray(bias, F32) * (scale * 8.0)
        return out

    BALLf = np.zeros((128, E), F32)
    BALLf[0:9] = baug(inputs["Bq"], inputs["bq"], 1.0 / 64.0)
    BALLf[32:41] = baug(inputs["Bk"], inputs["bk"], 1.0 / 8.0)
    BALLf[64:73] = baug(inputs["Bv"], inputs["bv"], 1.0 / 8.0)
    BALL = bf(BALLf)
    BoT = bf(baug(inputs["Bo"], inputs["bo"], 1.0 / 8.0))

    slopes = 0.5 ** np.arange(H, dtype=F32)
    jpos = np.arange(S, dtype=F32)
    Efull = np.exp(slopes[:, None] * (jpos[None, :] - (S - 1))).astype(F32)  # [H,S]
    rz_vec = np.full((128, 1), rez, F32)

    in_maps = []
    for c in range(NC):
        b, r = c // 4, c % 4
        sl = slice(TQ * r, TQ * (r + 1))
        m = {
            "xT": np.ascontiguousarray(x[b, sl, :].T),
            "WqT": WqT, "WkT": WkT, "WvT": WvT, "WoT": WoT,
            "AT": AT, "AoT": AoT,
            "BALL": BALL, "BoT": BoT,
            "ET": np.ascontiguousarray(Efull[:, sl].T),
            "rz": rz_vec,
        }
        in_maps.append(m)
    return in_maps


def kernel(**inputs) -> np.ndarray:
    from concourse import bass_utils

    nc = _get_built()
    in_maps = _prep_inputs(inputs)
    res = bass_utils.run_bass_kernel_spmd(nc, in_maps, core_ids=list(range(NC)))
    out = np.zeros((B, S, E), F32)
    for c in range(NC):
        b, r = c // 4, c % 4
        out[b, TQ * r:TQ * (r + 1), :] = res.results[c]["out"].T
    return out


if __name__ == "__main__":
    _get_built()
    print("build+compile OK")
